# revision 1
# baseline (speedup 1.0000x reference)
"""Trainium2 Bass kernel for nn_MoELayer (top-2 MoE, E=8 experts).

Strategy (expert-parallel across 8 NeuronCores):
  - Host computes the (tiny) gate matmul + top-2 + softmax, and dispatches
    each token to its two experts' cores ("all-to-all" done host-side as the
    sharding step). One expert per core.
  - Each core runs a Bass kernel computing, for its expert e and its routed
    tokens:   out = (silu(tok @ W1[e]) @ W2[e]) * gate_weight
    with bf16 matmul inputs and fp32 PSUM accumulation. Weights stay
    resident in SBUF; only the top-2-selected tokens are computed
    (4x fewer FLOPs than the dense reference).
  - Host scatter-adds the two weighted expert outputs per token.

Layouts (chosen so no on-device transposes are needed):
  stage 1:  actT[f, c] = silu( sum_d W1[d, f] * tokT[d, c] )
            matmul(lhsT=W1[dk, fj-tile], rhs=tokT[dk, c-chunk]) -> PSUM [f, c]
  stage 2:  out[c, d] = sum_f actT[f, c] * W2[f, d]
            matmul(lhsT=actT[fk, c-tile], rhs=W2[fk, d-chunk]) -> PSUM [c, d]

C (token capacity per core) is the exact max routed-token count, not
rounded up: stage 1 chunks may have non-multiple-of-128 widths and the
final stage-2 token tile may have <128 partitions.
"""

import math
import sys

sys.path.insert(0, "/opt/trn_rl_repo")

import ml_dtypes
import numpy as np

B, T, D, F, E = 2, 2048, 1024, 4096, 8
N = B * T
P = 128
KD = D // P  # 8
KF = F // P  # 32

bf16 = ml_dtypes.bfloat16

_nc_cache: dict[int, object] = {}
LAST_RESULTS = None  # BassKernelResults from the most recent run (for test.py)
TRACE = False


def _chunk_sizes(C: int) -> list[int]:
    """Split C into near-equal chunks of <=512 (stage-1 matmul free dim /
    PSUM bank limit), smallest first so the critical first token transfer
    is as small as possible."""
    n = math.ceil(C / 512)
    base = math.ceil(C / (n * P)) * P
    sizes = []
    rem = C
    while rem > 0:
        s = min(base, rem)
        sizes.append(s)
        rem -= s
    return sorted(sizes)


def _build(C: int):
    import concourse.mybir as mybir
    import concourse.tile as tile
    from concourse import bacc

    dt = mybir.dt

    nc = bacc.Bacc(None, target_bir_lowering=False)

    chunks = _chunk_sizes(C)

    # one token tensor per chunk -> fully contiguous per-partition DMA
    # packets (KD*cn*2 bytes) instead of 768B strided slices
    tokts = [
        nc.dram_tensor(f"tokt{i}", [P, KD, cn], dt.bfloat16, kind="ExternalInput")
        for i, cn in enumerate(chunks)
    ]
    w1 = nc.dram_tensor("w1", [P, KD, F], dt.bfloat16, kind="ExternalInput")
    w2 = nc.dram_tensor("w2", [P, KF, D], dt.bfloat16, kind="ExternalInput")
    # output is transposed: [D, C] with D on partitions; the gate-weight
    # scale + transpose happen on the host during scatter-add
    out = nc.dram_tensor("out", [D, C], dt.float32, kind="ExternalOutput")

    with tile.TileContext(nc) as tc:
        with (
            tc.tile_pool(name="const", bufs=1) as cpool,
            tc.tile_pool(name="act", bufs=1) as apool,
            tc.tile_pool(name="ps1", bufs=2, space="PSUM") as ps1pool,
            tc.tile_pool(name="ps2", bufs=2, space="PSUM") as ps2pool,
            tc.tile_pool(name="ob", bufs=4) as opool,
        ):
            w1_sb = cpool.tile([P, KD, F], dt.bfloat16, tag="w1")
            w2_sb = cpool.tile([P, KF, D], dt.bfloat16, tag="w2")
            tok_sbs = [
                cpool.tile(
                    [P, KD, cn], dt.bfloat16, tag=f"tok{i}", name=f"tok_sb{i}"
                )
                for i, cn in enumerate(chunks)
            ]

            # Input loads, all on the sync engine's HW DGE (SW DGE via other
            # engines measured far slower), emission-ordered by first use:
            # chunk-0 tokens, W1 quarters, remaining tokens, then W2.
            nc.sync.dma_start(tok_sbs[0][:], tokts[0][:])
            FQ = F // 4
            for q in range(4):
                nc.sync.dma_start(
                    w1_sb[:, :, q * FQ : (q + 1) * FQ],
                    w1[:, :, q * FQ : (q + 1) * FQ],
                )
            for i in range(1, len(chunks)):
                nc.sync.dma_start(tok_sbs[i][:], tokts[i][:])
            for q in range(4):
                nc.sync.dma_start(
                    w2_sb[:, q * (KF // 4) : (q + 1) * (KF // 4), :],
                    w2[:, q * (KF // 4) : (q + 1) * (KF // 4), :],
                )

            c0 = 0
            for ci, cn in enumerate(chunks):
                tok_sb = tok_sbs[ci]
                act_sb = apool.tile([P, KF, cn], dt.bfloat16, tag="act")
                # ---- stage 1: actT = silu(W1^T @ tokT) ----
                for fj in range(KF):
                    ps1 = ps1pool.tile([P, cn], dt.float32, tag="ps1")
                    for dk in range(KD):
                        nc.tensor.matmul(
                            ps1[:],
                            w1_sb[:, dk, fj * P : (fj + 1) * P],
                            tok_sb[:, dk, :],
                            start=(dk == 0),
                            stop=(dk == KD - 1),
                        )
                    nc.scalar.activation(
                        act_sb[:, fj, :],
                        ps1[:],
                        mybir.ActivationFunctionType.Silu,
                    )
                # ---- stage 2: outT = W2^T @ actT  (D on partitions,
                # tokens on the free dim -> no padded token tiles) ----
                for dm in range(D // P):
                    ps2 = ps2pool.tile([P, cn], dt.float32, tag="ps2")
                    for fk in range(KF):
                        nc.tensor.matmul(
                            ps2[:],
                            w2_sb[:, fk, dm * P : (dm + 1) * P],
                            act_sb[:, fk, :],
                            start=(fk == 0),
                            stop=(fk == KF - 1),
                        )
                    ob = opool.tile([P, cn], dt.float32, tag="ob")
                    nc.vector.tensor_copy(ob[:], ps2[:])
                    nc.sync.dma_start(
                        out[dm * P : (dm + 1) * P, c0 : c0 + cn],
                        ob[:],
                    )
                c0 += cn

    nc.compile()
    return nc


def _get_nc(C: int):
    if C not in _nc_cache:
        _nc_cache[C] = _build(C)
    return _nc_cache[C]


def kernel(**inputs) -> np.ndarray:
    global LAST_RESULTS
    x = np.asarray(inputs["x"], dtype=np.float32)
    Wg = np.asarray(inputs["Wg"], dtype=np.float32)
    W1 = np.asarray(inputs["W1"], dtype=np.float32)
    W2 = np.asarray(inputs["W2"], dtype=np.float32)

    h = np.ascontiguousarray(x.reshape(N, D))

    # ---- host gate: top-2 + softmax (0.05% of total FLOPs) ----
    logits = h @ Wg.T  # [N, E] f32
    idx2 = np.argpartition(-logits, 1, axis=1)[:, :2]
    lsel = np.take_along_axis(logits, idx2, axis=1)
    first = lsel[:, 0] >= lsel[:, 1]
    i0 = np.where(first, idx2[:, 0], idx2[:, 1])
    i1 = np.where(first, idx2[:, 1], idx2[:, 0])
    l0 = np.where(first, lsel[:, 0], lsel[:, 1])
    l1 = np.where(first, lsel[:, 1], lsel[:, 0])
    e1 = np.exp((l1 - l0).astype(np.float32))
    w0 = (1.0 / (1.0 + e1)).astype(np.float32)
    w1g = (e1 / (1.0 + e1)).astype(np.float32)

    token_ids = np.concatenate([np.arange(N), np.arange(N)])
    expert_ids = np.concatenate([i0, i1])
    gate_w = np.concatenate([w0, w1g])

    counts = np.bincount(expert_ids, minlength=E)
    C = int(counts.max())

    hb = h.astype(bf16)
    W1b = W1.astype(bf16)
    W2b = W2.astype(bf16)

    in_maps = []
    ids_per_expert = []
    gw_per_expert = []
    for e in range(E):
        sel = np.flatnonzero(expert_ids == e)
        ids_e = token_ids[sel]
        n_e = len(ids_e)
        ids_per_expert.append(ids_e)
        gw_per_expert.append(gate_w[sel])

        tokT = np.zeros((P, KD, C), dtype=bf16)
        # tokens [n,D] -> [D,n] -> [KD,P,n] -> [P,KD,n]
        tokT[:, :, :n_e] = (
            hb[ids_e].T.reshape(KD, P, n_e).transpose(1, 0, 2)
        )
        m = {
            "w1": np.ascontiguousarray(
                W1b[e].reshape(KD, P, F).transpose(1, 0, 2)
            ),
            "w2": np.ascontiguousarray(
                W2b[e].reshape(KF, P, D).transpose(1, 0, 2)
            ),
        }
        c0 = 0
        for i, cn in enumerate(_chunk_sizes(C)):
            m[f"tokt{i}"] = np.ascontiguousarray(tokT[:, :, c0 : c0 + cn])
            c0 += cn
        in_maps.append(m)

    nc = _get_nc(C)
    from concourse.bass_utils import run_bass_kernel_spmd

    LAST_RESULTS = run_bass_kernel_spmd(
        nc, in_maps, core_ids=list(range(E)), trace=TRACE
    )

    y = np.zeros((N, D), dtype=np.float32)
    for e in range(E):
        o = np.asarray(LAST_RESULTS.results[e]["out"], dtype=np.float32)  # [D, C]
        ids_e = ids_per_expert[e]
        n_e = len(ids_e)
        y[ids_e] += gw_per_expert[e][:, None] * o[:, :n_e].T
    return y.reshape(B, T, D)



# revision 2
# speedup vs baseline: 1.0439x; 1.0439x over previous
"""Trainium2 Bass kernel for nn_MoELayer (top-2 MoE, E=8 experts).

Strategy (expert-parallel across 8 NeuronCores):
  - Host computes the (tiny) gate matmul + top-2 + softmax, and dispatches
    each token to its two experts' cores ("all-to-all" done host-side as the
    sharding step). One expert per core.
  - Each core runs a Bass kernel computing, for its expert e and its routed
    tokens:   out = (silu(tok @ W1[e]) @ W2[e]) * gate_weight
    with bf16 matmul inputs and fp32 PSUM accumulation. Weights stay
    resident in SBUF; only the top-2-selected tokens are computed
    (4x fewer FLOPs than the dense reference).
  - Host scatter-adds the two weighted expert outputs per token.

Layouts (chosen so no on-device transposes are needed):
  stage 1:  actT[f, c] = silu( sum_d W1[d, f] * tokT[d, c] )
            matmul(lhsT=W1[dk, fj-tile], rhs=tokT[dk, c-chunk]) -> PSUM [f, c]
  stage 2:  out[c, d] = sum_f actT[f, c] * W2[f, d]
            matmul(lhsT=actT[fk, c-tile], rhs=W2[fk, d-chunk]) -> PSUM [c, d]

Weights live in DRAM as 4D tiles ([P, KF, KD, 128] / [P, KD, KF, 128]) so
that the fine-grained streaming slices below are fully contiguous 2KB+
per-partition DMA lines:
  - W1 is loaded in 32 per-fj slices (256KB each): the first matmul only
    waits for slice 0 + the first token chunk (~2us of DMA) instead of a
    2MB quarter, and the rest of W1 streams in under the stage-1 matmuls.
  - W2 is loaded in 8 per-dm slices interleaved with the remaining token
    chunks, each landing just before stage 2's dm-loop consumes it.

The first token chunk is ~256 cols: small enough to land fast, large
enough that stage 1's per-fj weight-slice consumption cadence (8 matmuls)
stays above the 256KB slice DMA time (no PE stall on the weight stream).
"""

import math
import sys

sys.path.insert(0, "/opt/trn_rl_repo")

import ml_dtypes
import numpy as np

B, T, D, F, E = 2, 2048, 1024, 4096, 8
N = B * T
P = 128
KD = D // P  # 8
KF = F // P  # 32

bf16 = ml_dtypes.bfloat16

_nc_cache: dict[int, object] = {}
LAST_RESULTS = None  # BassKernelResults from the most recent run (for test.py)
TRACE = False


def _chunk_sizes(C: int) -> list[int]:
    """First chunk ~256 (fast start, no weight-stream stall), 512s in the
    middle, remainder balanced at the end (also keeps the tail copy+DMA
    short)."""
    if C <= 512:
        return [C]
    sizes = [256]
    rem = C - 256
    while rem > 512 + 256:
        sizes.append(512)
        rem -= 512
    if rem > 512:
        sizes += [(rem + 1) // 2, rem - (rem + 1) // 2]
    else:
        sizes.append(rem)
    return sizes


def _build(C: int):
    import concourse.mybir as mybir
    import concourse.tile as tile
    from concourse import bacc

    dt = mybir.dt

    nc = bacc.Bacc(None, target_bir_lowering=False)

    chunks = _chunk_sizes(C)

    # one token tensor per chunk -> fully contiguous per-partition DMA
    # packets (KD*cn*2 bytes)
    tokts = [
        nc.dram_tensor(f"tokt{i}", [P, KD, cn], dt.bfloat16, kind="ExternalInput")
        for i, cn in enumerate(chunks)
    ]
    # w1[p, fj, dk, fi]  = W1[dk*P + p, fj*128 + fi]   (per-fj slice contiguous)
    w1 = nc.dram_tensor("w1", [P, KF, KD, P], dt.bfloat16, kind="ExternalInput")
    # w2[p, dm, fk, di]  = W2[fk*P + p, dm*128 + di]   (per-dm slice contiguous)
    w2 = nc.dram_tensor("w2", [P, KD, KF, P], dt.bfloat16, kind="ExternalInput")
    # output is transposed: [D, C] with D on partitions; the gate-weight
    # scale + transpose happen on the host during scatter-add
    out = nc.dram_tensor("out", [D, C], dt.float32, kind="ExternalOutput")

    with tile.TileContext(nc) as tc:
        with (
            tc.tile_pool(name="const", bufs=1) as cpool,
            tc.tile_pool(name="act", bufs=1) as apool,
            tc.tile_pool(name="ps1", bufs=2, space="PSUM") as ps1pool,
            tc.tile_pool(name="ps2", bufs=2, space="PSUM") as ps2pool,
            tc.tile_pool(name="ob", bufs=4) as opool,
        ):
            w1_sb = cpool.tile([P, KF, KD, P], dt.bfloat16, tag="w1")
            w2_sb = cpool.tile([P, KD, KF, P], dt.bfloat16, tag="w2")
            tok_sbs = [
                cpool.tile(
                    [P, KD, cn], dt.bfloat16, tag=f"tok{i}", name=f"tok_sb{i}"
                )
                for i, cn in enumerate(chunks)
            ]

            # Input loads, all on the sync engine's HW DGE (single queue,
            # transfers run in emission order). Ordered so the first matmul
            # waits only ~2us, and every later consumer's data lands just
            # ahead of its use while the PE stays busy:
            #   w1[fj=0], tok0, w1[fj=1..31], w2[dm=0], tok1, w2[dm=1],
            #   tok2, w2[dm=2..7]
            nc.sync.dma_start(w1_sb[:, 0], w1[:, 0])
            nc.sync.dma_start(tok_sbs[0][:], tokts[0][:])
            for fj in range(1, KF):
                nc.sync.dma_start(w1_sb[:, fj], w1[:, fj])
            toks_left = list(range(1, len(chunks)))
            for dm in range(KD):
                nc.sync.dma_start(w2_sb[:, dm], w2[:, dm])
                if dm < 2 and toks_left:
                    i = toks_left.pop(0)
                    nc.sync.dma_start(tok_sbs[i][:], tokts[i][:])
            for i in toks_left:
                nc.sync.dma_start(tok_sbs[i][:], tokts[i][:])

            c0 = 0
            for ci, cn in enumerate(chunks):
                tok_sb = tok_sbs[ci]
                act_sb = apool.tile([P, KF, cn], dt.bfloat16, tag="act")
                # ---- stage 1: actT = silu(W1^T @ tokT) ----
                for fj in range(KF):
                    ps1 = ps1pool.tile([P, cn], dt.float32, tag="ps1")
                    for dk in range(KD):
                        nc.tensor.matmul(
                            ps1[:],
                            w1_sb[:, fj, dk],
                            tok_sb[:, dk, :],
                            start=(dk == 0),
                            stop=(dk == KD - 1),
                        )
                    nc.scalar.activation(
                        act_sb[:, fj, :],
                        ps1[:],
                        mybir.ActivationFunctionType.Silu,
                    )
                # ---- stage 2: outT = W2^T @ actT  (D on partitions,
                # tokens on the free dim -> no padded token tiles) ----
                for dm in range(KD):
                    ps2 = ps2pool.tile([P, cn], dt.float32, tag="ps2")
                    for fk in range(KF):
                        nc.tensor.matmul(
                            ps2[:],
                            w2_sb[:, dm, fk],
                            act_sb[:, fk, :],
                            start=(fk == 0),
                            stop=(fk == KF - 1),
                        )
                    ob = opool.tile([P, cn], dt.float32, tag="ob")
                    nc.vector.tensor_copy(ob[:], ps2[:])
                    nc.sync.dma_start(
                        out[dm * P : (dm + 1) * P, c0 : c0 + cn],
                        ob[:],
                    )
                c0 += cn

    nc.compile()
    return nc


def _get_nc(C: int):
    if C not in _nc_cache:
        _nc_cache[C] = _build(C)
    return _nc_cache[C]


def kernel(**inputs) -> np.ndarray:
    global LAST_RESULTS
    x = np.asarray(inputs["x"], dtype=np.float32)
    Wg = np.asarray(inputs["Wg"], dtype=np.float32)
    W1 = np.asarray(inputs["W1"], dtype=np.float32)
    W2 = np.asarray(inputs["W2"], dtype=np.float32)

    h = np.ascontiguousarray(x.reshape(N, D))

    # ---- host gate: top-2 + softmax (0.05% of total FLOPs) ----
    logits = h @ Wg.T  # [N, E] f32
    idx2 = np.argpartition(-logits, 1, axis=1)[:, :2]
    lsel = np.take_along_axis(logits, idx2, axis=1)
    first = lsel[:, 0] >= lsel[:, 1]
    i0 = np.where(first, idx2[:, 0], idx2[:, 1])
    i1 = np.where(first, idx2[:, 1], idx2[:, 0])
    l0 = np.where(first, lsel[:, 0], lsel[:, 1])
    l1 = np.where(first, lsel[:, 1], lsel[:, 0])
    e1 = np.exp((l1 - l0).astype(np.float32))
    w0 = (1.0 / (1.0 + e1)).astype(np.float32)
    w1g = (e1 / (1.0 + e1)).astype(np.float32)

    token_ids = np.concatenate([np.arange(N), np.arange(N)])
    expert_ids = np.concatenate([i0, i1])
    gate_w = np.concatenate([w0, w1g])

    counts = np.bincount(expert_ids, minlength=E)
    C = int(counts.max())

    hb = h.astype(bf16)
    W1b = W1.astype(bf16)
    W2b = W2.astype(bf16)

    chunks = _chunk_sizes(C)
    in_maps = []
    ids_per_expert = []
    gw_per_expert = []
    for e in range(E):
        sel = np.flatnonzero(expert_ids == e)
        ids_e = token_ids[sel]
        n_e = len(ids_e)
        ids_per_expert.append(ids_e)
        gw_per_expert.append(gate_w[sel])

        tokT = np.zeros((P, KD, C), dtype=bf16)
        # tokens [n,D] -> [D,n] -> [KD,P,n] -> [P,KD,n]
        tokT[:, :, :n_e] = (
            hb[ids_e].T.reshape(KD, P, n_e).transpose(1, 0, 2)
        )
        m = {
            # [D, F] -> [KD, P, KF, 128] -> [P, KF, KD, 128]
            "w1": np.ascontiguousarray(
                W1b[e].reshape(KD, P, KF, P).transpose(1, 2, 0, 3)
            ),
            # [F, D] -> [KF, P, KD, 128] -> [P, KD, KF, 128]
            "w2": np.ascontiguousarray(
                W2b[e].reshape(KF, P, KD, P).transpose(1, 2, 0, 3)
            ),
        }
        c0 = 0
        for i, cn in enumerate(chunks):
            m[f"tokt{i}"] = np.ascontiguousarray(tokT[:, :, c0 : c0 + cn])
            c0 += cn
        in_maps.append(m)

    nc = _get_nc(C)
    from concourse.bass_utils import run_bass_kernel_spmd

    LAST_RESULTS = run_bass_kernel_spmd(
        nc, in_maps, core_ids=list(range(E)), trace=TRACE
    )

    y = np.zeros((N, D), dtype=np.float32)
    for e in range(E):
        o = np.asarray(LAST_RESULTS.results[e]["out"], dtype=np.float32)  # [D, C]
        ids_e = ids_per_expert[e]
        n_e = len(ids_e)
        y[ids_e] += gw_per_expert[e][:, None] * o[:, :n_e].T
    return y.reshape(B, T, D)


# revision 7
# speedup vs baseline: 1.0773x; 1.0319x over previous
"""Trainium2 Bass kernel for nn_MoELayer (top-2 MoE, E=8 experts).

Strategy (tensor-parallel over the FFN dim, 8 NeuronCores):
  - Host computes the (tiny) gate matmul + top-2 + softmax and groups the
    2N=8192 (token, expert) pairs by expert.
  - Every core processes ALL 8192 pairs, but only a 512-wide slice of the
    FFN dimension F: core c uses W1[:, :, c*512:(c+1)*512] and
    W2[:, c*512:(c+1)*512, :], producing a PARTIAL down-projection.
    Host sums the 8 partials, applies gate weights, and scatter-adds into
    the output. This is perfectly load-balanced by construction: the
    per-expert routing imbalance (max count 1071 vs mean 1024 for the
    graded input) costs nothing, unlike expert-parallel capacity padding.
  - Per core: 2 * 8192 * 1024 * 512 MACs = 524288 PE-streaming cycles
    (218.5us at 2.4GHz) -- the bf16 tensor-engine floor.

Within a core, pairs are processed in chunks of <=512 tokens (PSUM bank
limit), grouped by expert so each chunk uses one expert's weight tiles
(all 8 experts' F-slices stay SBUF-resident: 2 x 64KB/partition).

  stage 1:  actT[f, c] = silu( sum_d W1s[d, f] * tokT[d, c] )   f in [0,512)
  stage 2:  partT[d, c] = sum_f actT[f, c] * W2s[f, d]

Emission is software-pipelined (stage-1 of chunk j+1 before stage-2 of
chunk j) so stage-2 never waits on the silu latency of its own chunk's
last f-tile.

DMA (single sync-engine HW DGE queue, transfers run in emission order):
  - weights/tokens stream just-in-time: w1[e0] in 4 slices, tok chunks and
    w2/w1 per expert interleaved ahead of their consumers.
  - outputs are staged per-chunk ([P, KD, cn] bf16) and their descriptors
    are interleaved into the queue 3 expert-groups behind the inputs so
    they never head-of-line-block an input the PE is about to need.
  - the last chunk's output goes out per-dm-tile so the end-of-kernel
    exposed transfer is ~100KB, not ~1MB.
A burst of dependency-free garbage matmuls is emitted first: it runs
during the input-DMA wait and warms the PE HAM clock-gate (else the
first ~3.4us of real matmuls run at 1.2GHz instead of 2.4GHz).
"""

import math
import sys

sys.path.insert(0, "/opt/trn_rl_repo")

import ml_dtypes
import numpy as np

B, T, D, F, E = 2, 2048, 1024, 4096, 8
N = B * T
P = 128
KD = D // P  # 8
FS = F // E  # 512 F-slice per core
KFS = FS // P  # 4
CMAX = 512

bf16 = ml_dtypes.bfloat16

_nc_cache: dict[tuple, object] = {}
LAST_RESULTS = None  # BassKernelResults from the most recent run (for test.py)
TRACE = False


def _plan(counts) -> list[tuple[int, int]]:
    """Chunk plan: list of (expert, cn). Near-equal chunks of <=512 per
    expert; expert 0's first chunk is capped at ~384 so the very first
    matmul's token DMA is small (fast pipeline start)."""
    plan = []
    for e in range(E):
        n = int(counts[e])
        if n == 0:
            continue
        sizes = []
        if e == 0 and n > CMAX:
            sizes.append(384)
            n -= 384
        k = max(1, math.ceil(n / CMAX))
        base, extra = divmod(n, k)
        sizes += [base + 1] * extra + [base] * (k - extra)
        plan += [(e, cn) for cn in sizes if cn > 0]
    return plan


def _build(plan: tuple[tuple[int, int], ...]):
    import concourse.mybir as mybir
    import concourse.tile as tile
    from concourse import bacc

    dt = mybir.dt

    nc = bacc.Bacc(None, target_bir_lowering=False)

    nchunk = len(plan)
    tokts = [
        nc.dram_tensor(f"tokt{j}", [P, KD, cn], dt.bfloat16, kind="ExternalInput")
        for j, (e, cn) in enumerate(plan)
    ]
    # w1[p, e*KFS+fj, dk, fi] = W1slice[e][dk*P + p, fj*128 + fi]
    w1 = nc.dram_tensor("w1", [P, E * KFS, KD, P], dt.bfloat16, kind="ExternalInput")
    # w2[p, e*KD+dm, fk, di] = W2slice[e][fk*P + p, dm*128 + di]
    w2 = nc.dram_tensor("w2", [P, E * KD, KFS, P], dt.bfloat16, kind="ExternalInput")
    outs = [
        nc.dram_tensor(f"out{j}", [P, KD, cn], dt.bfloat16, kind="ExternalOutput")
        for j, (e, cn) in enumerate(plan)
    ]

    # chunk index ranges per expert group (for DMA interleaving)
    grp = [[] for _ in range(E)]
    for j, (e, cn) in enumerate(plan):
        grp[e].append(j)

    with tile.TileContext(nc) as tc:
        with (
            tc.tile_pool(name="const", bufs=1) as cpool,
            tc.tile_pool(name="tok", bufs=3) as tpool,
            tc.tile_pool(name="act", bufs=2) as apool,
            tc.tile_pool(name="stg", bufs=3) as spool,
            tc.tile_pool(name="ps1", bufs=2, space="PSUM") as ps1pool,
            tc.tile_pool(name="ps2", bufs=2, space="PSUM") as ps2pool,
            tc.tile_pool(name="warm", bufs=1, space="PSUM") as wpool,
        ):
            w1_sb = cpool.tile([P, E * KFS, KD, P], dt.bfloat16, tag="w1")
            w2_sb = cpool.tile([P, E * KD, KFS, P], dt.bfloat16, tag="w2")

            # ---- PE warm-up: matmuls on a zeroed scratch tile, no DMA
            # deps. ~3.5us of PE activity starting as soon as the engine
            # is free -> HAM un-throttles to 2.4GHz before the first real
            # matmul (and finishes before its input DMAs land). ----
            warm_sb = cpool.tile([P, 192], dt.bfloat16, tag="warm_sb")
            nc.vector.memset(warm_sb[:], 0)
            wps = wpool.tile([P, 64], dt.float32, tag="warm")
            for _ in range(40):
                nc.tensor.matmul(
                    wps[:], warm_sb[:, :128], warm_sb[:, 128:192],
                    start=True, stop=True, skip_group_check=True,
                )

            tok_sbs = {}
            act_sbs = {}
            stg_sbs = {}

            def load_tok(j):
                e, cn = plan[j]
                t = tpool.tile([P, KD, cn], dt.bfloat16, tag="tok",
                               name=f"tok_sb{j}")
                tok_sbs[j] = t
                nc.sync.dma_start(t[:], tokts[j][:])

            # ---- input DMA stream (emission order == transfer order) ----
            # expert 0: w1 slice fj=0 first, first tok chunk, rest of w1
            nc.sync.dma_start(w1_sb[:, 0], w1[:, 0])
            load_tok(grp[0][0])
            for fj in range(1, KFS):
                nc.sync.dma_start(w1_sb[:, fj], w1[:, fj])
            for j in grp[0][1:]:
                load_tok(j)
            nc.sync.dma_start(w2_sb[:, 0:KD], w2[:, 0:KD])
            for e in range(1, 3):
                nc.sync.dma_start(
                    w1_sb[:, e * KFS : (e + 1) * KFS], w1[:, e * KFS : (e + 1) * KFS]
                )
                for j in grp[e]:
                    load_tok(j)
                nc.sync.dma_start(
                    w2_sb[:, e * KD : (e + 1) * KD], w2[:, e * KD : (e + 1) * KD]
                )

            out_emitted = 0

            def emit_outs(upto):
                nonlocal out_emitted
                while out_emitted < upto:
                    j = out_emitted
                    e, cn = plan[j]
                    if j == nchunk - 1:
                        for dm in range(KD):
                            nc.sync.dma_start(
                                outs[j][:, dm, :], stg_sbs[j][:, dm, :]
                            )
                    else:
                        nc.sync.dma_start(outs[j][:], stg_sbs[j][:])
                    del stg_sbs[j]
                    out_emitted += 1

            def emit_inputs(e):
                nc.sync.dma_start(
                    w1_sb[:, e * KFS : (e + 1) * KFS], w1[:, e * KFS : (e + 1) * KFS]
                )
                for j in grp[e]:
                    load_tok(j)
                nc.sync.dma_start(
                    w2_sb[:, e * KD : (e + 1) * KD], w2[:, e * KD : (e + 1) * KD]
                )

            # remaining input groups + lagging output descriptors are
            # emitted inside the compute loop below (so the queue stays
            # input-ahead / output-behind).

            def stage1(j):
                e, cn = plan[j]
                tok_sb = tok_sbs.pop(j)
                a = apool.tile([P, KFS, cn], dt.bfloat16, tag="act",
                               name=f"act_sb{j}")
                act_sbs[j] = a
                for fj in range(KFS):
                    ps1 = ps1pool.tile([P, cn], dt.float32, tag="ps1")
                    for dk in range(KD):
                        nc.tensor.matmul(
                            ps1[:],
                            w1_sb[:, e * KFS + fj, dk],
                            tok_sb[:, dk, :],
                            start=(dk == 0),
                            stop=(dk == KD - 1),
                        )
                    nc.scalar.activation(
                        a[:, fj, :], ps1[:], mybir.ActivationFunctionType.Silu
                    )

            def stage2(j):
                e, cn = plan[j]
                a = act_sbs.pop(j)
                stg = spool.tile([P, KD, cn], dt.bfloat16, tag="stg",
                                 name=f"stg_sb{j}")
                stg_sbs[j] = stg
                for dm in range(KD):
                    ps2 = ps2pool.tile([P, cn], dt.float32, tag="ps2")
                    for fk in range(KFS):
                        nc.tensor.matmul(
                            ps2[:],
                            w2_sb[:, e * KD + dm, fk],
                            a[:, fk, :],
                            start=(fk == 0),
                            stop=(fk == KFS - 1),
                        )
                    nc.vector.tensor_copy(stg[:, dm, :], ps2[:])

            # ---- software-pipelined chunk loop ----
            # order: s1(0), s1(1), s2(0), s1(2), s2(1), ... s2(last)
            # inputs for expert group e land 3 groups ahead of use;
            # output descriptors trail ~1 group behind production.
            next_in_grp = 3
            stage1(0)
            for j in range(1, nchunk):
                e_j = plan[j][0]
                while next_in_grp <= min(e_j + 2, E - 1):
                    emit_inputs(next_in_grp)
                    next_in_grp += 1
                stage1(j)
                stage2(j - 1)
                emit_outs(j - 1)
            while next_in_grp < E:
                emit_inputs(next_in_grp)
                next_in_grp += 1
            stage2(nchunk - 1)
            emit_outs(nchunk)

    nc.compile()
    return nc


def _get_nc(plan):
    key = tuple(plan)
    if key not in _nc_cache:
        _nc_cache[key] = _build(key)
    return _nc_cache[key]


def kernel(**inputs) -> np.ndarray:
    global LAST_RESULTS
    x = np.asarray(inputs["x"], dtype=np.float32)
    Wg = np.asarray(inputs["Wg"], dtype=np.float32)
    W1 = np.asarray(inputs["W1"], dtype=np.float32)
    W2 = np.asarray(inputs["W2"], dtype=np.float32)

    h = np.ascontiguousarray(x.reshape(N, D))

    # ---- host gate: top-2 + softmax (0.05% of total FLOPs) ----
    logits = h @ Wg.T  # [N, E] f32
    idx2 = np.argpartition(-logits, 1, axis=1)[:, :2]
    lsel = np.take_along_axis(logits, idx2, axis=1)
    first = lsel[:, 0] >= lsel[:, 1]
    i0 = np.where(first, idx2[:, 0], idx2[:, 1])
    i1 = np.where(first, idx2[:, 1], idx2[:, 0])
    l0 = np.where(first, lsel[:, 0], lsel[:, 1])
    l1 = np.where(first, lsel[:, 1], lsel[:, 0])
    e1 = np.exp((l1 - l0).astype(np.float32))
    w0 = (1.0 / (1.0 + e1)).astype(np.float32)
    w1g = (e1 / (1.0 + e1)).astype(np.float32)

    token_ids = np.concatenate([np.arange(N), np.arange(N)])
    expert_ids = np.concatenate([i0, i1])
    gate_w = np.concatenate([w0, w1g])

    counts = np.bincount(expert_ids, minlength=E)
    plan = _plan(counts)

    hb = h.astype(bf16)
    W1b = W1.astype(bf16)
    W2b = W2.astype(bf16)

    # pair order: grouped by expert (matches the chunk plan)
    order_pairs = np.concatenate(
        [np.flatnonzero(expert_ids == e) for e in range(E)]
    )
    tids = token_ids[order_pairs]
    gws = gate_w[order_pairs]

    # tokens [2N,D] -> [D,2N] -> [KD,P,2N] -> [P,KD,2N], then chunked
    tokT = np.ascontiguousarray(
        hb[tids].T.reshape(KD, P, 2 * N).transpose(1, 0, 2)
    )
    tok_chunks = {}
    c0 = 0
    for j, (e, cn) in enumerate(plan):
        tok_chunks[f"tokt{j}"] = np.ascontiguousarray(tokT[:, :, c0 : c0 + cn])
        c0 += cn
    assert c0 == 2 * N

    in_maps = []
    for c in range(E):
        # core c's F-slice of every expert's weights
        w1p = np.stack(
            [
                # [D, FS] -> [KD, P, KFS, 128] -> [P, KFS, KD, 128]
                W1b[e][:, c * FS : (c + 1) * FS]
                .reshape(KD, P, KFS, P)
                .transpose(1, 2, 0, 3)
                for e in range(E)
            ],
            axis=1,
        ).reshape(P, E * KFS, KD, P)
        w2p = np.stack(
            [
                # [FS, D] -> [KFS, P, KD, 128] -> [P, KD, KFS, 128]
                W2b[e][c * FS : (c + 1) * FS, :]
                .reshape(KFS, P, KD, P)
                .transpose(1, 2, 0, 3)
                for e in range(E)
            ],
            axis=1,
        ).reshape(P, E * KD, KFS, P)
        m = {"w1": np.ascontiguousarray(w1p), "w2": np.ascontiguousarray(w2p)}
        m.update(tok_chunks)
        in_maps.append(m)

    nc = _get_nc(plan)
    from concourse.bass_utils import run_bass_kernel_spmd

    LAST_RESULTS = run_bass_kernel_spmd(
        nc, in_maps, core_ids=list(range(E)), trace=TRACE
    )

    # ---- combine: sum the 8 partial outputs, gate-weight, pair-reduce ----
    Ot = np.zeros((P, KD, 2 * N), dtype=np.float32)
    for c in range(E):
        c0 = 0
        for j, (e, cn) in enumerate(plan):
            Ot[:, :, c0 : c0 + cn] += np.asarray(
                LAST_RESULTS.results[c][f"out{j}"], dtype=np.float32
            )
            c0 += cn
    # Ot[p, dm, col] = partial_out[dm*128+p, col] -> [2N, D]
    contrib = Ot.transpose(2, 1, 0).reshape(2 * N, D)
    contrib *= gws[:, None]
    srt = np.argsort(tids, kind="stable")
    cs = contrib[srt]
    y = cs[0::2] + cs[1::2]
    return y.reshape(B, T, D)


# revision 9
# speedup vs baseline: 1.0813x; 1.0037x over previous
"""Trainium2 Bass kernel for nn_MoELayer (top-2 MoE, E=8 experts).

Strategy (tensor-parallel over the FFN dim, 8 NeuronCores):
  - Host computes the (tiny) gate matmul + top-2 + softmax and groups the
    2N=8192 (token, expert) pairs by expert.
  - Every core processes ALL 8192 pairs, but only a 512-wide slice of the
    FFN dimension F: core c uses W1[:, :, c*512:(c+1)*512] and
    W2[:, c*512:(c+1)*512, :], producing a PARTIAL down-projection.
    Host sums the 8 partials, applies gate weights, and scatter-adds into
    the output. This is perfectly load-balanced by construction: the
    per-expert routing imbalance (max count 1071 vs mean 1024 for the
    graded input) costs nothing, unlike expert-parallel capacity padding.
  - Per core: 2 * 8192 * 1024 * 512 MACs = 524288 PE-streaming cycles
    (218.5us at 2.4GHz) -- the bf16 tensor-engine floor.

Within a core, pairs are processed in chunks of <=512 tokens (PSUM bank
limit), grouped by expert so each chunk uses one expert's weight tiles
(all 8 experts' F-slices stay SBUF-resident: 2 x 64KB/partition).

  stage 1:  actT[f, c] = silu( sum_d W1s[d, f] * tokT[d, c] )   f in [0,512)
  stage 2:  partT[d, c] = sum_f actT[f, c] * W2s[f, d]

Emission is software-pipelined (stage-1 of chunk j+1 before stage-2 of
chunk j) so stage-2 never waits on the silu latency of its own chunk's
last f-tile.

DMA (single sync-engine HW DGE queue, transfers run in emission order):
  - weights/tokens stream just-in-time: w1[e0] in 4 slices, tok chunks and
    w2/w1 per expert interleaved ahead of their consumers.
  - outputs are staged per-chunk ([P, KD, cn] bf16) and their descriptors
    are interleaved into the queue 3 expert-groups behind the inputs so
    they never head-of-line-block an input the PE is about to need.
  - the last chunk's output goes out per-dm-tile so the end-of-kernel
    exposed transfer is ~100KB, not ~1MB.
A burst of dependency-free garbage matmuls is emitted first: it runs
during the input-DMA wait and warms the PE HAM clock-gate (else the
first ~3.4us of real matmuls run at 1.2GHz instead of 2.4GHz).
"""

import math
import sys

sys.path.insert(0, "/opt/trn_rl_repo")

import ml_dtypes
import numpy as np

B, T, D, F, E = 2, 2048, 1024, 4096, 8
N = B * T
P = 128
KD = D // P  # 8
FS = F // E  # 512 F-slice per core
KFS = FS // P  # 4
CMAX = 512

bf16 = ml_dtypes.bfloat16

_nc_cache: dict[tuple, object] = {}
LAST_RESULTS = None  # BassKernelResults from the most recent run (for test.py)
TRACE = False


def _plan(counts) -> list[tuple[int, int]]:
    """Chunk plan: list of (expert, cn). Near-equal chunks of <=512 per
    expert; expert 0's first chunk is capped at ~384 so the very first
    matmul's token DMA is small (fast pipeline start)."""
    plan = []
    for e in range(E):
        n = int(counts[e])
        if n == 0:
            continue
        sizes = []
        if e == 0 and n > CMAX:
            sizes.append(256)
            n -= 256
        k = max(1, math.ceil(n / CMAX))
        base, extra = divmod(n, k)
        sizes += [base + 1] * extra + [base] * (k - extra)
        plan += [(e, cn) for cn in sizes if cn > 0]
    return plan


def _build(plan: tuple[tuple[int, int], ...]):
    import concourse.mybir as mybir
    import concourse.tile as tile
    from concourse import bacc

    dt = mybir.dt

    nc = bacc.Bacc(None, target_bir_lowering=False)

    nchunk = len(plan)
    tokts = [
        nc.dram_tensor(f"tokt{j}", [P, KD, cn], dt.bfloat16, kind="ExternalInput")
        for j, (e, cn) in enumerate(plan)
    ]
    # w1[p, e*KFS+fj, dk, fi] = W1slice[e][dk*P + p, fj*128 + fi]
    w1 = nc.dram_tensor("w1", [P, E * KFS, KD, P], dt.bfloat16, kind="ExternalInput")
    # w2[p, e*KD+dm, fk, di] = W2slice[e][fk*P + p, dm*128 + di]
    w2 = nc.dram_tensor("w2", [P, E * KD, KFS, P], dt.bfloat16, kind="ExternalInput")
    outs = [
        nc.dram_tensor(f"out{j}", [P, KD, cn], dt.bfloat16, kind="ExternalOutput")
        for j, (e, cn) in enumerate(plan)
    ]

    # chunk index ranges per expert group (for DMA interleaving)
    grp = [[] for _ in range(E)]
    for j, (e, cn) in enumerate(plan):
        grp[e].append(j)

    with tile.TileContext(nc) as tc:
        with (
            tc.tile_pool(name="const", bufs=1) as cpool,
            tc.tile_pool(name="tok", bufs=3) as tpool,
            tc.tile_pool(name="act", bufs=2) as apool,
            tc.tile_pool(name="stg", bufs=3) as spool,
            tc.tile_pool(name="ps1", bufs=2, space="PSUM") as ps1pool,
            tc.tile_pool(name="ps2", bufs=2, space="PSUM") as ps2pool,
            tc.tile_pool(name="warm", bufs=1, space="PSUM") as wpool,
        ):
            w1_sb = cpool.tile([P, E * KFS, KD, P], dt.bfloat16, tag="w1")
            w2_sb = cpool.tile([P, E * KD, KFS, P], dt.bfloat16, tag="w2")

            # ---- PE warm-up: matmuls on a zeroed scratch tile, no DMA
            # deps. ~3.5us of PE activity starting as soon as the engine
            # is free -> HAM un-throttles to 2.4GHz before the first real
            # matmul (and finishes before its input DMAs land). ----
            warm_sb = cpool.tile([P, 192], dt.bfloat16, tag="warm_sb")
            nc.vector.memset(warm_sb[:], 0)
            wps = wpool.tile([P, 64], dt.float32, tag="warm")
            for _ in range(120):
                nc.tensor.matmul(
                    wps[:], warm_sb[:, :128], warm_sb[:, 128:192],
                    start=True, stop=True, skip_group_check=True,
                )

            tok_sbs = {}
            act_sbs = {}
            stg_sbs = {}

            def load_tok(j):
                e, cn = plan[j]
                t = tpool.tile([P, KD, cn], dt.bfloat16, tag="tok",
                               name=f"tok_sb{j}")
                tok_sbs[j] = t
                nc.sync.dma_start(t[:], tokts[j][:])

            # ---- input DMA stream (emission order == transfer order) ----
            # expert 0: w1 slice fj=0 first, first tok chunk, rest of w1
            nc.sync.dma_start(w1_sb[:, 0], w1[:, 0])
            load_tok(grp[0][0])
            for fj in range(1, KFS):
                nc.sync.dma_start(w1_sb[:, fj], w1[:, fj])
            for j in grp[0][1:]:
                load_tok(j)
            nc.sync.dma_start(w2_sb[:, 0:KD], w2[:, 0:KD])
            for e in range(1, 3):
                nc.sync.dma_start(
                    w1_sb[:, e * KFS : (e + 1) * KFS], w1[:, e * KFS : (e + 1) * KFS]
                )
                for j in grp[e]:
                    load_tok(j)
                nc.sync.dma_start(
                    w2_sb[:, e * KD : (e + 1) * KD], w2[:, e * KD : (e + 1) * KD]
                )

            out_emitted = 0

            def emit_outs(upto):
                nonlocal out_emitted
                while out_emitted < upto:
                    j = out_emitted
                    e, cn = plan[j]
                    if j == nchunk - 1:
                        for dm in range(KD):
                            nc.sync.dma_start(
                                outs[j][:, dm, :], stg_sbs[j][:, dm, :]
                            )
                    else:
                        nc.sync.dma_start(outs[j][:], stg_sbs[j][:])
                    del stg_sbs[j]
                    out_emitted += 1

            def emit_inputs(e):
                nc.sync.dma_start(
                    w1_sb[:, e * KFS : (e + 1) * KFS], w1[:, e * KFS : (e + 1) * KFS]
                )
                for j in grp[e]:
                    load_tok(j)
                nc.sync.dma_start(
                    w2_sb[:, e * KD : (e + 1) * KD], w2[:, e * KD : (e + 1) * KD]
                )

            # remaining input groups + lagging output descriptors are
            # emitted inside the compute loop below (so the queue stays
            # input-ahead / output-behind).

            def stage1(j):
                e, cn = plan[j]
                tok_sb = tok_sbs.pop(j)
                a = apool.tile([P, KFS, cn], dt.bfloat16, tag="act",
                               name=f"act_sb{j}")
                act_sbs[j] = a
                for fj in range(KFS):
                    ps1 = ps1pool.tile([P, cn], dt.float32, tag="ps1")
                    for dk in range(KD):
                        nc.tensor.matmul(
                            ps1[:],
                            w1_sb[:, e * KFS + fj, dk],
                            tok_sb[:, dk, :],
                            start=(dk == 0),
                            stop=(dk == KD - 1),
                        )
                    nc.scalar.activation(
                        a[:, fj, :], ps1[:], mybir.ActivationFunctionType.Silu
                    )

            def stage2(j):
                e, cn = plan[j]
                a = act_sbs.pop(j)
                stg = spool.tile([P, KD, cn], dt.bfloat16, tag="stg",
                                 name=f"stg_sb{j}")
                stg_sbs[j] = stg
                for dm in range(KD):
                    ps2 = ps2pool.tile([P, cn], dt.float32, tag="ps2")
                    for fk in range(KFS):
                        nc.tensor.matmul(
                            ps2[:],
                            w2_sb[:, e * KD + dm, fk],
                            a[:, fk, :],
                            start=(fk == 0),
                            stop=(fk == KFS - 1),
                        )
                    nc.vector.tensor_copy(stg[:, dm, :], ps2[:])

            # ---- software-pipelined chunk loop ----
            # order: s1(0), s1(1), s2(0), s1(2), s2(1), ... s2(last)
            # inputs for expert group e land 3 groups ahead of use;
            # output descriptors trail ~1 group behind production.
            next_in_grp = 3
            stage1(0)
            for j in range(1, nchunk):
                e_j = plan[j][0]
                while next_in_grp <= min(e_j + 2, E - 1):
                    emit_inputs(next_in_grp)
                    next_in_grp += 1
                stage1(j)
                stage2(j - 1)
                emit_outs(j - 1)
            while next_in_grp < E:
                emit_inputs(next_in_grp)
                next_in_grp += 1
            stage2(nchunk - 1)
            emit_outs(nchunk)

    nc.compile()
    return nc


def _get_nc(plan):
    key = tuple(plan)
    if key not in _nc_cache:
        _nc_cache[key] = _build(key)
    return _nc_cache[key]


def kernel(**inputs) -> np.ndarray:
    global LAST_RESULTS
    x = np.asarray(inputs["x"], dtype=np.float32)
    Wg = np.asarray(inputs["Wg"], dtype=np.float32)
    W1 = np.asarray(inputs["W1"], dtype=np.float32)
    W2 = np.asarray(inputs["W2"], dtype=np.float32)

    h = np.ascontiguousarray(x.reshape(N, D))

    # ---- host gate: top-2 + softmax (0.05% of total FLOPs) ----
    logits = h @ Wg.T  # [N, E] f32
    idx2 = np.argpartition(-logits, 1, axis=1)[:, :2]
    lsel = np.take_along_axis(logits, idx2, axis=1)
    first = lsel[:, 0] >= lsel[:, 1]
    i0 = np.where(first, idx2[:, 0], idx2[:, 1])
    i1 = np.where(first, idx2[:, 1], idx2[:, 0])
    l0 = np.where(first, lsel[:, 0], lsel[:, 1])
    l1 = np.where(first, lsel[:, 1], lsel[:, 0])
    e1 = np.exp((l1 - l0).astype(np.float32))
    w0 = (1.0 / (1.0 + e1)).astype(np.float32)
    w1g = (e1 / (1.0 + e1)).astype(np.float32)

    token_ids = np.concatenate([np.arange(N), np.arange(N)])
    expert_ids = np.concatenate([i0, i1])
    gate_w = np.concatenate([w0, w1g])

    counts = np.bincount(expert_ids, minlength=E)
    plan = _plan(counts)

    hb = h.astype(bf16)
    W1b = W1.astype(bf16)
    W2b = W2.astype(bf16)

    # pair order: grouped by expert (matches the chunk plan)
    order_pairs = np.concatenate(
        [np.flatnonzero(expert_ids == e) for e in range(E)]
    )
    tids = token_ids[order_pairs]
    gws = gate_w[order_pairs]

    # tokens [2N,D] -> [D,2N] -> [KD,P,2N] -> [P,KD,2N], then chunked
    tokT = np.ascontiguousarray(
        hb[tids].T.reshape(KD, P, 2 * N).transpose(1, 0, 2)
    )
    tok_chunks = {}
    c0 = 0
    for j, (e, cn) in enumerate(plan):
        tok_chunks[f"tokt{j}"] = np.ascontiguousarray(tokT[:, :, c0 : c0 + cn])
        c0 += cn
    assert c0 == 2 * N

    in_maps = []
    for c in range(E):
        # core c's F-slice of every expert's weights
        w1p = np.stack(
            [
                # [D, FS] -> [KD, P, KFS, 128] -> [P, KFS, KD, 128]
                W1b[e][:, c * FS : (c + 1) * FS]
                .reshape(KD, P, KFS, P)
                .transpose(1, 2, 0, 3)
                for e in range(E)
            ],
            axis=1,
        ).reshape(P, E * KFS, KD, P)
        w2p = np.stack(
            [
                # [FS, D] -> [KFS, P, KD, 128] -> [P, KD, KFS, 128]
                W2b[e][c * FS : (c + 1) * FS, :]
                .reshape(KFS, P, KD, P)
                .transpose(1, 2, 0, 3)
                for e in range(E)
            ],
            axis=1,
        ).reshape(P, E * KD, KFS, P)
        m = {"w1": np.ascontiguousarray(w1p), "w2": np.ascontiguousarray(w2p)}
        m.update(tok_chunks)
        in_maps.append(m)

    nc = _get_nc(plan)
    from concourse.bass_utils import run_bass_kernel_spmd

    LAST_RESULTS = run_bass_kernel_spmd(
        nc, in_maps, core_ids=list(range(E)), trace=TRACE
    )

    # ---- combine: sum the 8 partial outputs, gate-weight, pair-reduce ----
    Ot = np.zeros((P, KD, 2 * N), dtype=np.float32)
    for c in range(E):
        c0 = 0
        for j, (e, cn) in enumerate(plan):
            Ot[:, :, c0 : c0 + cn] += np.asarray(
                LAST_RESULTS.results[c][f"out{j}"], dtype=np.float32
            )
            c0 += cn
    # Ot[p, dm, col] = partial_out[dm*128+p, col] -> [2N, D]
    contrib = Ot.transpose(2, 1, 0).reshape(2 * N, D)
    contrib *= gws[:, None]
    srt = np.argsort(tids, kind="stable")
    cs = contrib[srt]
    y = cs[0::2] + cs[1::2]
    return y.reshape(B, T, D)
